# revision 8
# baseline (speedup 1.0000x reference)
"""Trainium2 Bass kernel for nn_CustomLoss (argmax-distance weighted loss).

reference:
    arg = argmax(target, axis=1)              # [B]
    delta = distance[arg]                     # [B]
    err = |distance[None,:] - delta[:,None]| + 1
    loss = sum((output - target) * err) / B

Math (no gathers, data-parallel over 8 NeuronCores):
  With dist = [-0.5, -0.34, 0, 0.34, 0.5] and e_a = [t_a >= max_c t_c]:
    w2 = 2*delta = (e4 + 0.68*e3) - (e0 + 0.68*e1)
    wI[b,c] = |w2 - 2*dist_c| = 2*(err - 1)
    loss*2B = sum(wI * d) + 2*sum(d),  d = o - t

Layout: rows on 128 partitions, 5 classes interleaved along free dim,
8 tiles of [128, 2560] per core.  Both inputs stream in as bf16 via
SWDGE cast-DMA (GpSimd does DMA descriptor-gen ONLY -- gpsimd compute
stalls concurrent DVE ops via the shared SBUF port).  The 2*dist_c
pattern D2 arrives as a DMA'd input (memsets cost ~7us of startup).

Per-iteration emission is ordered so DVE keeps streaming during ACT
latency: reduce_k, d_k run while ACT replicates W2R_{k-1}; then
z_{k-1}, q1_{k-1}; then is_ge_k after ACT's mC_k is ready.
  DVE:    reduce (1x) | is_ge, z, d, q1 (2x bf16 TT) | w2-combine smalls
  ScalarE: mC, W2R broadcasts + wI = |z|
  TensorE: ones-matmul sums of q1 and d into single-bank PSUM accums
Output: [1, 2*G] f32 per core; host computes (sum_q + 2*sum_d)/(2B).
"""

from contextlib import ExitStack

import numpy as np

P = 128
C = 5
DIST = (-0.5, -0.34, 0.0, 0.34, 0.5)
B = 4194304
NCORES = 8
ROWS_PER_CORE = B // NCORES  # 524288
G = 512                      # rows per partition per tile
NTILES = ROWS_PER_CORE // (P * G)  # 8
FREE = C * G

_CACHE = {}


def _d2_host_array():
    import ml_dtypes

    pat = np.tile(np.asarray([2.0 * v for v in DIST], np.float32), G)  # [FREE]
    return np.ascontiguousarray(
        np.broadcast_to(pat, (P, FREE)).astype(ml_dtypes.bfloat16)
    )


def _build_nc():
    import concourse.bacc as bacc
    import concourse.mybir as mybir
    import concourse.tile as tile

    F32 = mybir.dt.float32
    BF16 = mybir.dt.bfloat16

    nc = bacc.Bacc(target_bir_lowering=False)

    t_in = nc.declare_dram_parameter("t", [ROWS_PER_CORE, C], F32, isOutput=False)
    o_in = nc.declare_dram_parameter("o", [ROWS_PER_CORE, C], F32, isOutput=False)
    d2_in = nc.declare_dram_parameter("d2", [P, FREE], BF16, isOutput=False)
    out = nc.declare_dram_parameter("out", [1, 2 * G], F32, isOutput=True)

    # row = n*(P*G) + p*G + g ; per-partition data is contiguous in DRAM
    t_tiled = t_in.rearrange("(n p g) c -> n p (g c)", p=P, g=G)
    o_tiled = o_in.rearrange("(n p g) c -> n p (g c)", p=P, g=G)

    ones_bf16 = nc.const_aps.aps[(BF16, 1.0)]  # [128, 1] of 1.0, preregistered

    with ExitStack() as ctx:
        tc = ctx.enter_context(tile.TileContext(nc))
        pool = ctx.enter_context(tc.tile_pool(name="work", bufs=3))
        sp = ctx.enter_context(tc.tile_pool(name="small", bufs=4))
        accp = ctx.enter_context(tc.tile_pool(name="acc", bufs=1))
        psp = ctx.enter_context(tc.tile_pool(name="ps", bufs=1, space="PSUM"))

        d2t = accp.tile([P, FREE], BF16, name="d2t")
        nc.sync.dma_start(d2t[:, :], d2_in[:, :])

        ps_q = psp.tile([1, G], F32, name="ps_q")  # accumulated sums of q1
        ps_d = psp.tile([1, G], F32, name="ps_d")  # accumulated sums of d

        state = {}

        def emit_loads(k):
            t = pool.tile([P, FREE], BF16, tag="t", name="t", bufs=3)
            nc.gpsimd.dma_start(t[:, :], t_tiled[k])  # f32 -> bf16 cast in DMA
            o = pool.tile([P, FREE], BF16, tag="o", name="o", bufs=3)
            nc.gpsimd.dma_start(o[:, :], o_tiled[k])
            return t, o

        def emit_stage1(k, t, o):
            """reduce + d: DVE work independent of ScalarE results."""
            tv = t[:, :].rearrange("p (g c) -> p g c", c=C)
            m = sp.tile([P, G], BF16, tag="m", name="m", bufs=4)
            nc.vector.tensor_reduce(
                m[:, :], tv, axis=mybir.AxisListType.X, op=mybir.AluOpType.max
            )
            # mC = m replicated x5 (ScalarE broadcast copy)
            mC = pool.tile([P, FREE], BF16, tag="mC", name="mC", bufs=3)
            nc.scalar.copy(mC[:, :], m[:, :].to_broadcast([P, G, C]))
            d = pool.tile([P, FREE], BF16, tag="d", name="d", bufs=3)
            nc.vector.tensor_sub(d[:, :], o[:, :], t[:, :])
            return mC, d

        def emit_stage2(k, t, mC, d):
            """is_ge + w2 combine."""
            E = pool.tile([P, FREE], BF16, tag="E", name="E", bufs=3)
            nc.vector.tensor_tensor(
                E[:, :], t[:, :], mC[:, :], op=mybir.AluOpType.is_ge
            )
            Ev = E[:, :].rearrange("p (g c) -> p g c", c=C)
            # a2 = 0.68*e3 + e4 ; b2 = 0.68*e1 + e0 ; w2 = a2 - b2
            a2 = sp.tile([P, G], BF16, tag="a2", name="a2", bufs=4)
            nc.vector.scalar_tensor_tensor(
                a2[:, :], Ev[:, :, 3], 0.68, Ev[:, :, 4],
                mybir.AluOpType.mult, mybir.AluOpType.add,
            )
            b2 = sp.tile([P, G], BF16, tag="b2", name="b2", bufs=4)
            nc.vector.scalar_tensor_tensor(
                b2[:, :], Ev[:, :, 1], 0.68, Ev[:, :, 0],
                mybir.AluOpType.mult, mybir.AluOpType.add,
            )
            w2 = sp.tile([P, G], BF16, tag="w2", name="w2", bufs=4)
            nc.vector.tensor_sub(w2[:, :], a2[:, :], b2[:, :])
            state[k] = (w2, d)

        def emit_back_a(k):
            """W2R replicate (ACT) + z (DVE) + wI (ACT)."""
            w2, d = state.pop(k)
            W2R = pool.tile([P, FREE], BF16, tag="W2R", name="W2R", bufs=3)
            nc.scalar.copy(W2R[:, :], w2[:, :].to_broadcast([P, G, C]))
            z = pool.tile([P, FREE], BF16, tag="z", name="z", bufs=3)
            nc.vector.tensor_sub(z[:, :], W2R[:, :], d2t[:, :])
            wI = pool.tile([P, FREE], BF16, tag="wI", name="wI", bufs=3)
            nc.scalar.activation(
                wI[:, :], z[:, :], mybir.ActivationFunctionType.Abs
            )
            return wI, d

        def emit_back_b(k, wI, d):
            """q1 product + TensorE sums."""
            q1 = pool.tile([P, FREE], BF16, tag="q1", name="q1", bufs=3)
            nc.vector.tensor_mul(q1[:, :], wI[:, :], d[:, :])
            for j in range(C):
                first = k == 0 and j == 0
                last = k == NTILES - 1 and j == C - 1
                sl = slice(j * G, (j + 1) * G)
                nc.tensor.matmul(
                    ps_q[:, :], ones_bf16, q1[:, sl], start=first, stop=last
                )
                nc.tensor.matmul(
                    ps_d[:, :], ones_bf16, d[:, sl], start=first, stop=last
                )

        def emit_back_final(k):
            """Last tile: back-phase in pipelined half-tiles so the ACT->DVE
            chain drains in ~half the time (nothing overlaps it otherwise)."""
            w2, d = state.pop(k)
            H = G // 2  # rows per half; free-dim half is H*C wide
            W2R = pool.tile([P, FREE], BF16, tag="W2R", name="W2Rf", bufs=3)
            z = pool.tile([P, FREE], BF16, tag="z", name="zf", bufs=3)
            wI = pool.tile([P, FREE], BF16, tag="wI", name="wIf", bufs=3)
            q1 = pool.tile([P, FREE], BF16, tag="q1", name="q1f", bufs=3)
            for h in range(2):
                fsl = slice(h * H * C, (h + 1) * H * C)
                w2h = w2[:, h * H : (h + 1) * H]
                nc.scalar.copy(W2R[:, fsl], w2h.to_broadcast([P, H, C]))
                nc.vector.tensor_sub(z[:, fsl], W2R[:, fsl], d2t[:, fsl])
                nc.scalar.activation(
                    wI[:, fsl], z[:, fsl], mybir.ActivationFunctionType.Abs
                )
                nc.vector.tensor_mul(q1[:, fsl], wI[:, fsl], d[:, fsl])
                for j in range(C):
                    first = False
                    last = h == 1 and j == C - 1
                    sl = slice(j * G + h * H, j * G + (h + 1) * H)
                    nc.tensor.matmul(
                        ps_q[:, 0:H], ones_bf16, q1[:, sl], start=first, stop=last
                    )
                    nc.tensor.matmul(
                        ps_d[:, 0:H], ones_bf16, d[:, sl], start=first, stop=last
                    )

        # Software pipeline: loads lead by one tile; within iteration k the
        # DVE order is [reduce_k, d_k | z_{k-1}, q1_{k-1} | is_ge_k, combines]
        # so DVE streams while ScalarE produces mC_k / W2R_{k-1} / wI_{k-1}.
        tiles = {}
        tiles[0] = emit_loads(0)
        back = None
        for k in range(NTILES):
            if k + 1 < NTILES:
                tiles[k + 1] = emit_loads(k + 1)
            t, o = tiles.pop(k)
            mC, d = emit_stage1(k, t, o)
            if back is not None:
                emit_back_b(k - 1, *back)
                back = None
            if k >= 1:
                back = emit_back_a(k - 1)
            emit_stage2(k, t, mC, d)
            if back is not None:
                emit_back_b(k - 1, *back)
                back = None
        back = emit_back_a(NTILES - 1)
        emit_back_b(NTILES - 1, *back)

        # readout: PSUM -> SBUF -> DRAM; host computes (sum_q + 2*sum_d)/2B
        res = accp.tile([1, 2 * G], F32, name="res")
        nc.scalar.copy(res[:, 0:G], ps_q[:, :])
        nc.scalar.copy(res[:, G : 2 * G], ps_d[:, :])
        nc.sync.dma_start(out[:, :], res[:, :])
    nc.finalize()
    return nc


def _get_nc():
    if "nc" not in _CACHE:
        _CACHE["nc"] = _build_nc()
    return _CACHE["nc"]


def kernel(output, target, distance, _want_results=False):
    from concourse.bass_utils import run_bass_kernel_spmd

    output = np.asarray(output, dtype=np.float32)
    target = np.asarray(target, dtype=np.float32)
    distance = np.asarray(distance, dtype=np.float32)
    assert output.shape == (B, C) and target.shape == (B, C)
    assert np.allclose(distance, np.asarray(DIST, np.float32)), distance

    nc = _get_nc()
    d2 = _d2_host_array()
    o_sh = output.reshape(NCORES, ROWS_PER_CORE, C)
    t_sh = target.reshape(NCORES, ROWS_PER_CORE, C)
    in_maps = [
        {
            "t": np.ascontiguousarray(t_sh[i]),
            "o": np.ascontiguousarray(o_sh[i]),
            "d2": d2,
        }
        for i in range(NCORES)
    ]
    res = run_bass_kernel_spmd(nc, in_maps, core_ids=list(range(NCORES)))
    total = 0.0
    for r in res.results:
        arr = r["out"].astype(np.float64).reshape(2, G)
        total += float(arr[0].sum() + 2.0 * arr[1].sum())
    loss = np.float32(total / 2.0 / B)
    if _want_results:
        return loss, res
    return loss


# revision 10
# speedup vs baseline: 1.0182x; 1.0182x over previous
"""Trainium2 Bass kernel for nn_CustomLoss (argmax-distance weighted loss).

reference:
    arg = argmax(target, axis=1)              # [B]
    delta = distance[arg]                     # [B]
    err = |distance[None,:] - delta[:,None]| + 1
    loss = sum((output - target) * err) / B

Math (no gathers, data-parallel over 8 NeuronCores):
  With dist = [-0.5, -0.34, 0, 0.34, 0.5] and e_a = [t_a >= max_c t_c]:
    w2 = 2*delta = (e4 + 0.68*e3) - (e0 + 0.68*e1)
    wI[b,c] = |w2 - 2*dist_c| = 2*(err - 1)
    loss*2B = sum(wI * d) + 2*sum(d),  d = o - t

Layout: rows on 128 partitions, 5 classes interleaved along free dim,
8 tiles of [128, 2560] per core.  Both inputs stream in as bf16 via
SWDGE cast-DMA (GpSimd does DMA descriptor-gen ONLY -- gpsimd compute
stalls concurrent DVE ops via the shared SBUF port).  The 2*dist_c
pattern D2 arrives as a DMA'd input (memsets cost ~7us of startup).

Per-iteration emission is ordered so DVE keeps streaming during ACT
latency: reduce_k, d_k run while ACT replicates W2R_{k-1}; then
z_{k-1}, q1_{k-1}; then is_ge_k after ACT's mC_k is ready.
  DVE:    reduce (1x) | is_ge, z, d, q1 (2x bf16 TT) | w2-combine smalls
  ScalarE: mC, W2R broadcasts + wI = |z|
  TensorE: ones-matmul sums of q1 and d into single-bank PSUM accums
Output: [1, 2*G] f32 per core; host computes (sum_q + 2*sum_d)/(2B).
"""

from contextlib import ExitStack

import numpy as np

P = 128
C = 5
DIST = (-0.5, -0.34, 0.0, 0.34, 0.5)
B = 4194304
NCORES = 8
ROWS_PER_CORE = B // NCORES  # 524288
G = 512                      # rows per partition per tile
NTILES = ROWS_PER_CORE // (P * G)  # 8
FREE = C * G

_CACHE = {}


def _d2_host_array():
    import ml_dtypes

    pat = np.tile(np.asarray([2.0 * v for v in DIST], np.float32), G)  # [FREE]
    return np.ascontiguousarray(
        np.broadcast_to(pat, (P, FREE)).astype(ml_dtypes.bfloat16)
    )


def _build_nc():
    import concourse.bacc as bacc
    import concourse.mybir as mybir
    import concourse.tile as tile

    F32 = mybir.dt.float32
    BF16 = mybir.dt.bfloat16

    nc = bacc.Bacc(target_bir_lowering=False)

    t_in = nc.declare_dram_parameter("t", [ROWS_PER_CORE, C], F32, isOutput=False)
    o_in = nc.declare_dram_parameter("o", [ROWS_PER_CORE, C], F32, isOutput=False)
    d2_in = nc.declare_dram_parameter("d2", [P, FREE], BF16, isOutput=False)
    out = nc.declare_dram_parameter("out", [1, 2 * G], F32, isOutput=True)

    # row = n*(P*G) + p*G + g ; per-partition data is contiguous in DRAM
    t_tiled = t_in.rearrange("(n p g) c -> n p (g c)", p=P, g=G)
    o_tiled = o_in.rearrange("(n p g) c -> n p (g c)", p=P, g=G)

    ones_bf16 = nc.const_aps.aps[(BF16, 1.0)]  # [128, 1] of 1.0, preregistered

    with ExitStack() as ctx:
        tc = ctx.enter_context(tile.TileContext(nc))
        pool = ctx.enter_context(tc.tile_pool(name="work", bufs=3))
        sp = ctx.enter_context(tc.tile_pool(name="small", bufs=4))
        accp = ctx.enter_context(tc.tile_pool(name="acc", bufs=1))
        psp = ctx.enter_context(tc.tile_pool(name="ps", bufs=1, space="PSUM"))

        d2t = accp.tile([P, FREE], BF16, name="d2t")

        ps_q = psp.tile([1, G], F32, name="ps_q")  # accumulated sums of q1
        ps_d = psp.tile([1, G], F32, name="ps_d")  # accumulated sums of d

        state = {}

        def emit_loads(k):
            t = pool.tile([P, FREE], BF16, tag="t", name="t", bufs=3)
            nc.gpsimd.dma_start(t[:, :], t_tiled[k])  # f32 -> bf16 cast in DMA
            o = pool.tile([P, FREE], BF16, tag="o", name="o", bufs=3)
            nc.gpsimd.dma_start(o[:, :], o_tiled[k])
            return t, o

        def emit_stage1(k, t, o):
            """reduce + d: DVE work independent of ScalarE results."""
            tv = t[:, :].rearrange("p (g c) -> p g c", c=C)
            m = sp.tile([P, G], BF16, tag="m", name="m", bufs=4)
            nc.vector.tensor_reduce(
                m[:, :], tv, axis=mybir.AxisListType.X, op=mybir.AluOpType.max
            )
            # mC = m replicated x5 (ScalarE broadcast copy)
            mC = pool.tile([P, FREE], BF16, tag="mC", name="mC", bufs=3)
            nc.scalar.copy(mC[:, :], m[:, :].to_broadcast([P, G, C]))
            d = pool.tile([P, FREE], BF16, tag="d", name="d", bufs=3)
            nc.vector.tensor_sub(d[:, :], o[:, :], t[:, :])
            return mC, d

        def emit_stage2(k, t, mC, d):
            """is_ge + w2 combine."""
            E = pool.tile([P, FREE], BF16, tag="E", name="E", bufs=3)
            nc.vector.tensor_tensor(
                E[:, :], t[:, :], mC[:, :], op=mybir.AluOpType.is_ge
            )
            Ev = E[:, :].rearrange("p (g c) -> p g c", c=C)
            # a2 = 0.68*e3 + e4 ; b2 = 0.68*e1 + e0 ; w2 = a2 - b2
            a2 = sp.tile([P, G], BF16, tag="a2", name="a2", bufs=4)
            nc.vector.scalar_tensor_tensor(
                a2[:, :], Ev[:, :, 3], 0.68, Ev[:, :, 4],
                mybir.AluOpType.mult, mybir.AluOpType.add,
            )
            b2 = sp.tile([P, G], BF16, tag="b2", name="b2", bufs=4)
            nc.vector.scalar_tensor_tensor(
                b2[:, :], Ev[:, :, 1], 0.68, Ev[:, :, 0],
                mybir.AluOpType.mult, mybir.AluOpType.add,
            )
            w2 = sp.tile([P, G], BF16, tag="w2", name="w2", bufs=4)
            nc.vector.tensor_sub(w2[:, :], a2[:, :], b2[:, :])
            state[k] = (w2, d)

        def emit_back_a(k):
            """W2R replicate (ACT) + z (DVE) + wI (ACT)."""
            w2, d = state.pop(k)
            W2R = pool.tile([P, FREE], BF16, tag="W2R", name="W2R", bufs=3)
            nc.scalar.copy(W2R[:, :], w2[:, :].to_broadcast([P, G, C]))
            z = pool.tile([P, FREE], BF16, tag="z", name="z", bufs=3)
            nc.vector.tensor_sub(z[:, :], W2R[:, :], d2t[:, :])
            wI = pool.tile([P, FREE], BF16, tag="wI", name="wI", bufs=3)
            nc.scalar.activation(
                wI[:, :], z[:, :], mybir.ActivationFunctionType.Abs
            )
            return wI, d

        def emit_back_b(k, wI, d):
            """q1 product + TensorE sums (FD-512; f32 out must fit 1 PSUM bank)."""
            q1 = pool.tile([P, FREE], BF16, tag="q1", name="q1", bufs=3)
            nc.vector.tensor_mul(q1[:, :], wI[:, :], d[:, :])
            for j in range(C):
                first = k == 0 and j == 0
                last = k == NTILES - 1 and j == C - 1
                sl = slice(j * G, (j + 1) * G)
                nc.tensor.matmul(
                    ps_q[:, 0:G], ones_bf16, q1[:, sl], start=first, stop=last
                )
                nc.tensor.matmul(
                    ps_d[:, 0:G], ones_bf16, d[:, sl], start=first, stop=last
                )


        # Software pipeline: loads lead by one tile; within iteration k the
        # DVE order is [reduce_k, d_k | z_{k-1}, q1_{k-1} | is_ge_k, combines]
        # so DVE streams while ScalarE produces mC_k / W2R_{k-1} / wI_{k-1}.
        tiles = {}
        tiles[0] = emit_loads(0)
        nc.gpsimd.dma_start(d2t[:, :], d2_in[:, :])
        back = None
        for k in range(NTILES):
            if k + 1 < NTILES:
                tiles[k + 1] = emit_loads(k + 1)
            t, o = tiles.pop(k)
            mC, d = emit_stage1(k, t, o)
            if back is not None:
                emit_back_b(k - 1, *back)
                back = None
            if k >= 1:
                back = emit_back_a(k - 1)
            emit_stage2(k, t, mC, d)
            if back is not None:
                emit_back_b(k - 1, *back)
                back = None
        back = emit_back_a(NTILES - 1)
        emit_back_b(NTILES - 1, *back)

        # readout: PSUM -> SBUF -> DRAM; host computes (sum_q + 2*sum_d)/2B
        res = accp.tile([1, 2 * G], F32, name="res")
        nc.scalar.copy(res[:, 0:G], ps_q[:, :])
        nc.scalar.copy(res[:, G : 2 * G], ps_d[:, :])
        nc.sync.dma_start(out[:, :], res[:, :])
    nc.finalize()
    return nc


def _get_nc():
    if "nc" not in _CACHE:
        _CACHE["nc"] = _build_nc()
    return _CACHE["nc"]


def kernel(output, target, distance, _want_results=False):
    from concourse.bass_utils import run_bass_kernel_spmd

    output = np.asarray(output, dtype=np.float32)
    target = np.asarray(target, dtype=np.float32)
    distance = np.asarray(distance, dtype=np.float32)
    assert output.shape == (B, C) and target.shape == (B, C)
    assert np.allclose(distance, np.asarray(DIST, np.float32)), distance

    nc = _get_nc()
    d2 = _d2_host_array()
    o_sh = output.reshape(NCORES, ROWS_PER_CORE, C)
    t_sh = target.reshape(NCORES, ROWS_PER_CORE, C)
    in_maps = [
        {
            "t": np.ascontiguousarray(t_sh[i]),
            "o": np.ascontiguousarray(o_sh[i]),
            "d2": d2,
        }
        for i in range(NCORES)
    ]
    res = run_bass_kernel_spmd(nc, in_maps, core_ids=list(range(NCORES)))
    total = 0.0
    for r in res.results:
        arr = r["out"].astype(np.float64).reshape(2, G)
        total += float(arr[0].sum() + 2.0 * arr[1].sum())
    loss = np.float32(total / 2.0 / B)
    if _want_results:
        return loss, res
    return loss
